# revision 1
# baseline (speedup 1.0000x reference)
"""BiLSTM seq2seq + Bahdanau attention + vocab softmax on 8 trn2 NeuronCores.

Strategy (one uniform SPMD program; all divergence lives in per-core input data):
  - encoder fwd LSTM on cores 0-3, bwd on cores 4-7 (bwd cores receive
    time-reversed token indices; downstream attention is order-blind in s,
    so the scan-order storage never needs re-reversal)
  - pairwise AllGather exchanges the two encoder halves
  - decoder LSTM replicated on all cores (per-step cost is weight-streaming
    bound into the PE and independent of batch, so replication is free
    parallelism; collectives have a ~20us latency floor so per-step
    tensor-parallel sync is impossible)
  - attention token-sharded 8 ways; softmax-normalization of attention is
    deferred and folded into the output-dense scaling (per-partition scalar)
  - output dense vocab-sharded 8 ways in bf16; vocab softmax via one
    AllReduce of per-token partial sums

Recurrence matmuls run with the weight tile stationary and h^T streaming
(z lands as [gate-dim-on-partitions, batch] so gate nonlinearities are
full-width engine ops). The recurrent weights are fp8(e4m3), host-scaled by
SC=64 so N(0, 0.02^2) entries land in e4m3's normal range; FWL then loads
stationary tiles at 4 elem/lane/cycle, halving the weight-ingest bound vs
bf16. The 1/SC unscale is folded into the gate activations' scale field.
Gate tiles are packed position-major (m-tile 4j+q = gate q of state chunk j)
so each state chunk's gates finish together; the per-chunk elementwise then
pipelines under the remaining chunks' matmuls and the next step's k=j matmul
can start as soon as chunk j's h is written.
"""

import os
import numpy as np
import ml_dtypes
from contextlib import ExitStack

import concourse.bass as bass
import concourse.tile as tile
from concourse import mybir
from concourse.bass_utils import run_bass_kernel_spmd
from concourse.masks import make_identity

FP32 = mybir.dt.float32
BF16 = mybir.dt.bfloat16
FP8 = mybir.dt.float8e4
I32 = mybir.dt.int32
AF = mybir.ActivationFunctionType
ALU = mybir.AluOpType
ENG = mybir.EngineType

NC = 8
B = 4
TIN = 128
TOUT = 128
E = 512
H = 512
D = 2 * H            # 1024
V = 32000
VSH = V // NC        # 4000
TPC = TOUT // NC     # 16 token-positions per core
NTOK = B * TOUT      # 512 (b, t) pairs
TOKC = NTOK // NC    # 64 tokens per core
EM = E // 128        # 4 chunks of the embedding dim
KM = H // 128        # 4 K-chunks (encoder recurrence)
KD = D // 128        # 8 K-chunks (decoder recurrence)
ME = 4 * H // 128    # 16 gate m-tiles (encoder)
MD = 4 * D // 128    # 32 gate m-tiles (decoder)
NV = 8               # vocab free-chunks per core (500-wide: matmul out must fit one PSUM bank)
VW = VSH // NV       # 500
AGR = D + 8          # allgather rows: 1024 attn + row 1024 = denom + pad
SC = 64.0            # fp8 weight prescale (folded back out in activations)
HDT = mybir.dt.bfloat16  # h-stream dtype (PE allows fp8-weight x bf16-moving)
ENC_GROUPS = 2       # encoder state chunks processed per elementwise group
DEC_GROUPS = 2       # decoder groups


def sq(ap):
    """Merge trailing count-1 free dims (shape-match helper)."""
    n = len(ap.ap) - 1  # free dims
    names = " ".join(f"a{i}" for i in range(n))
    merged = f"a0 ({' '.join(f'a{i}' for i in range(1, n))})"
    return ap.rearrange(f"p {names} -> p {merged}")


def legalize_waits(nc, max_waits=1):
    """This walrus build accepts at most `max_waits` sync-wait commands per
    instruction; hoist excess waits onto injected same-engine NoOps."""
    n = 0

    def make_nop(engine, wait):
        eng = nc.engines[engine]
        inst = eng.nop(nofuse=True).ins
        bb = nc.cur_bb.bb
        lst = bb.instructions
        assert lst and lst[-1].name == inst.name
        lst.pop()
        bb.instructions = lst
        inst.sync_info = mybir.SyncInfo(on_wait=[wait], on_update=[])
        return inst

    for blk in nc.main_func.blocks:
        new_insts = []
        changed = False
        for inst in blk.instructions:
            si = inst.sync_info
            waits = list(si.on_wait) if si and si.on_wait else []
            if len(waits) > max_waits:
                excess, keep = waits[:-max_waits], waits[-max_waits:]
                for w in excess:
                    new_insts.append(make_nop(inst.engine, w))
                    n += 1
                si.on_wait = keep
                changed = True
            new_insts.append(inst)
        if changed:
            blk.instructions = new_insts
    return n


def build_program(debug=False, enc_unroll=4, dec_unroll=2, enc_steps=TIN,
                  dec_steps=TOUT, reps=1, static_loops=False,
                  stub_collectives=False):
    nc = bass.Bass("TRN2", target_bir_lowering=False, debug=False,
                   num_devices=NC)

    def din(name, shape, dt=FP32):
        return nc.dram_tensor(name, shape, dt, kind="ExternalInput").ap()

    def dout(name, shape, dt=FP32):
        return nc.dram_tensor(name, shape, dt, kind="ExternalOutput").ap()

    enc_mini = din("enc_mini", [NTOK, E])
    enc_idx = din("enc_idx", [128, EM], I32)
    dec_mini = din("dec_mini", [NTOK, E])
    dec_idx = din("dec_idx", [128, EM], I32)
    wx_m = din("wx_m", [E, 4 * H], BF16)
    wh_m = din("wh_m", [H, 4 * H], FP8)
    b_m = din("b_m", [128, ME])
    wx_d = din("wx_d", [E, 4 * D], BF16)
    wh_d = din("wh_d", [D, 4 * D], FP8)
    b_d = din("b_d", [128, MD])
    v_sc = din("v_sc", [128, KD], BF16)
    wo_sh = din("wo_sh", [D, VSH], BF16)

    o_probs = dout("o_probs", [NTOK, VSH])
    if debug:
        o_enc = dout("o_enc", [128, 2, KM, B, TIN])
        o_dec = dout("o_dec", [128, KD, B, TOUT])
        o_attn = dout("o_attn", [NC, AGR, TOKC])

    def collective(kind, op, ins, outs, groups):
        nc.gpsimd.collective_compute(kind, op, ins=ins, outs=outs,
                                     replica_groups=groups)

    with tile.TileContext(nc) as tc:
        # whole-run pools
        const = tc.alloc_tile_pool(name="const", bufs=1)
        work = tc.alloc_tile_pool(name="work", bufs=4)
        dram = tc.alloc_tile_pool(name="dram", bufs=1, space="DRAM")

        ident = const.tile([128, 128], FP32)
        make_identity(nc, ident[:])
        ones_col = const.tile([128, 1], BF16)
        nc.vector.memset(ones_col[:], 1.0)
        bm_sb = const.tile([128, ME], FP32)
        nc.sync.dma_start(bm_sb[:], b_m[:])
        bd_sb = const.tile([128, MD], FP32)
        nc.sync.dma_start(bd_sb[:], b_d[:])
        v_sb = const.tile([128, KD], BF16)
        nc.sync.dma_start(v_sb[:], v_sc[:])

        # encoder-lifetime + decoder-lifetime pools
        dec_w = tc.alloc_tile_pool(name="dec_w", bufs=1)
        enc_w = tc.alloc_tile_pool(name="enc_w", bufs=1)
        whm_sb = enc_w.tile([128, KM, 4 * H], FP8)
        whd_sb = dec_w.tile([128, KD, 4 * D], FP8)
        xw_m = enc_w.tile([128, ME, B, TIN], BF16)
        xw_d = dec_w.tile([128, MD, B, TOUT], BF16)

        # ---------------- phase 0: gathers + input projections -----------
        ph0 = tc.alloc_tile_pool(name="ph0", bufs=1)
        ph0w = tc.alloc_tile_pool(name="ph0w", bufs=3)
        ph0p = tc.alloc_tile_pool(name="ph0p", bufs=2, space="PSUM")
        # bulk-preload the input-projection weights: per-tile DMAs have
        # ~1.3us latency each and throttle the PE loop
        wxm_sb = ph0.tile([128, EM, 4 * H], BF16)
        nc.sync.dma_start(
            wxm_sb[:], wx_m[:].rearrange("(k p) g -> p k g", p=128))
        wxd_sb = ph0.tile([128, EM, 4 * D], BF16)
        nc.sync.dma_start(
            wxd_sb[:], wx_d[:].rearrange("(k p) g -> p k g", p=128))

        def gather_transpose(mini, idx_dram, xt_tile, idx_name):
            idx_sb = ph0.tile([128, EM], I32, name=idx_name)
            nc.sync.dma_start(idx_sb[:], idx_dram[:])
            for j in range(EM):  # 128-row batches of (b, t) rows
                rows = ph0w.tile([128, E], FP32, tag="gatrows")
                nc.gpsimd.indirect_dma_start(
                    out=rows[:], out_offset=None,
                    in_=mini[:],
                    in_offset=bass.IndirectOffsetOnAxis(
                        ap=idx_sb[:, j:j + 1], axis=0))
                for ech in range(EM):
                    tp = ph0p.tile([128, 128], FP32, tag="tp0")
                    nc.tensor.transpose(
                        out=tp[:], in_=rows[:, ech * 128:(ech + 1) * 128],
                        identity=ident[:])
                    nc.vector.tensor_copy(
                        xt_tile[:, ech, j * 128:(j + 1) * 128], tp[:])

        def project(wx_sb, xt_tile, nm, b_sb, xw_tile):
            # xw = SC * (x @ Wx + b); host passes b pre-scaled by SC
            for m in range(nm):
                pj = ph0p.tile([128, NTOK], FP32, tag="pj")
                for kblk in range(EM):
                    nc.tensor.matmul(
                        pj[:], wx_sb[:, kblk, m * 128:(m + 1) * 128],
                        xt_tile[:, kblk, :],
                        start=(kblk == 0), stop=(kblk == EM - 1))
                nc.scalar.activation(
                    xw_tile[:, m, :, :].rearrange("p b t -> p (b t)"),
                    pj[:], AF.Identity, bias=b_sb[:, m:m + 1], scale=SC)

        enc_xT = ph0.tile([128, EM, NTOK], BF16)
        gather_transpose(enc_mini, enc_idx, enc_xT, "idx_e")
        dec_xT = ph0.tile([128, EM, NTOK], BF16)
        gather_transpose(dec_mini, dec_idx, dec_xT, "idx_d")
        # recurrence weights load behind the gather-critical DMAs (they are
        # not needed until the loops start)
        nc.sync.dma_start(
            whm_sb[:], wh_m[:].rearrange("(k p) g -> p k g", p=128))
        nc.sync.dma_start(
            whd_sb[:], wh_d[:].rearrange("(k p) g -> p k g", p=128))
        project(wxm_sb, enc_xT, ME, bm_sb, xw_m)
        project(wxd_sb, dec_xT, MD, bd_sb, xw_d)

        ph0p.release()
        ph0w.release()
        ph0.release()

        # ---------------- phase 1: encoder recurrence ---------------------
        ench = tc.alloc_tile_pool(name="ench", bufs=1)
        recp = tc.alloc_tile_pool(name="recp", bufs=2, space="PSUM")
        enc_half = ench.tile([128, KM, B, TIN], FP32)
        # h is double-buffered (ping-pong by step parity): with a single
        # buffer the h-write has a WAR hazard against every matmul of its own
        # step, so the gate elementwise can never hide under the PE block.
        h_enc = [ench.tile([128, KM, B], HDT, name=f"h_enc{i}")
                 for i in range(2)]
        c_enc = ench.tile([128, KM, B], FP32)
        nc.vector.memset(h_enc[0][:], 0.0)
        nc.vector.memset(c_enc[:], 0.0)

        def lstm_step(km, groups, wh_sb, xw_src, xw_off, h_in, h_out,
                      c_st, out_dst):
            # position-major gate packing: m-tile 4j+q = gate q (i,f,o,g)
            # of state chunk j; process `groups` groups of cs chunks each.
            # No dynamic APs here — the unrolled body prefetches its xw
            # window and stages its h outputs with one dynamic DMA each
            # (per-step ds(iv) expressions exhaust engine registers).
            cs = km // groups
            # k-split ordering: all m-tiles consume the EARLY h chunks
            # first, so the step only needs the previous step's last
            # elementwise group right before its own last k-pass — the
            # group-chain latency hides under the early-k matmuls.
            pss = []
            for _g in range(groups):
                ps = recp.tile([128, 4 * cs, B], FP32, tag=f"rec_ps{_g}")
                pss.append(ps)
            for kg in range(groups):
                for g in range(groups):
                    for jj in range(cs):
                        for q in range(4):
                            m = 4 * (g * cs + jj) + q
                            for k in range(kg * cs, (kg + 1) * cs):
                                nc.tensor.matmul(
                                    pss[g][:, 4 * jj + q, :],
                                    wh_sb[:, k, m * 128:(m + 1) * 128],
                                    h_in[:, k, :],
                                    start=(k == 0), stop=(k == km - 1))
            for g in range(groups):
                j0 = g * cs
                ps = pss[g]
                z = work.tile([128, 4 * cs, B], FP32, tag="rec_z")
                nc.vector.tensor_tensor(
                    out=z[:], in0=ps[:],
                    in1=sq(xw_src[:, 4 * j0:4 * (j0 + cs), :,
                                  xw_off:xw_off + 1]),
                    op=ALU.add)
                zv = z[:].rearrange("p (c q) b -> p c q b", q=4)
                sio = work.tile([128, cs, 3, B], FP32, tag="rec_sio")
                tg = work.tile([128, cs, 1, B], FP32, tag="rec_tg")
                nc.scalar.activation(sio[:], zv[:, :, 0:3, :], AF.Sigmoid,
                                     scale=1.0 / SC)
                nc.scalar.activation(tg[:], zv[:, :, 3:4, :], AF.Tanh,
                                     scale=1.0 / SC)
                nc.vector.tensor_tensor(out=tg[:], in0=sio[:, :, 0:1, :],
                                        in1=tg[:], op=ALU.mult)
                cj = c_st[:, j0:j0 + cs, :]
                nc.vector.tensor_tensor(
                    out=cj, in0=cj,
                    in1=sq(sio[:, :, 1:2, :]), op=ALU.mult)
                nc.vector.tensor_tensor(out=cj, in0=cj, in1=sq(tg[:]),
                                        op=ALU.add)
                tc_t = work.tile([128, cs, B], FP32, tag="rec_tc")
                nc.scalar.activation(tc_t[:], cj, AF.Tanh)
                nc.vector.tensor_tensor(
                    out=h_out[:, j0:j0 + cs, :], in0=sq(sio[:, :, 2:3, :]),
                    in1=tc_t[:], op=ALU.mult)
            nc.vector.tensor_copy(out_dst, h_out[:])

        def rec_body(iv0, unroll, km, groups, wh_sb, xw, h_pair, c_st,
                     out_tile, nm, hook=None):
            # prefetch this body's xw window + stage h outputs; exactly one
            # dynamic AP on ACT (prefetch) and one on DVE (flush copy).
            # hook(i) interleaves extra work (attention tanh) between steps.
            if unroll == 1 and isinstance(iv0, int):
                lstm_step(km, groups, wh_sb, xw, iv0, h_pair[iv0 % 2],
                          h_pair[1 - iv0 % 2], c_st,
                          sq(out_tile[:, :, :, iv0:iv0 + 1]))
                if hook is not None:
                    hook(0)
                return
            xww = work.tile([128, nm, B, unroll], BF16, tag=f"xww{km}")
            nc.scalar.copy(xww[:], xw[:, :, :, bass.ds(iv0, unroll)])
            stage = work.tile([128, km, B, unroll], FP32, tag=f"stg{km}")
            for i in range(unroll):
                lstm_step(km, groups, wh_sb, xww, i, h_pair[i % 2],
                          h_pair[1 - i % 2], c_st, stage[:, :, :, i])
                if hook is not None:
                    hook(i)
            nc.vector.tensor_copy(out_tile[:, :, :, bass.ds(iv0, unroll)],
                                  stage[:])

        def enc_loop():
            if static_loops:
                for i in range(enc_steps):
                    rec_body(i, 1, KM, ENC_GROUPS, whm_sb, xw_m, h_enc,
                             c_enc, enc_half, ME)
            else:
                assert enc_unroll % 2 == 0
                tc.For_i_unrolled_general(
                    0, enc_steps, 1,
                    lambda iv0, unroll: rec_body(
                        iv0, unroll, KM, ENC_GROUPS, whm_sb, xw_m, h_enc,
                        c_enc, enc_half, ME),
                    max_unroll=enc_unroll, hint_engines=(ENG.PE,))
        if reps == 1:
            enc_loop()
        else:
            with tc.For_i(0, reps, 1):
                enc_loop()

        # ---------------- phase 2: exchange encoder halves ----------------
        # Two collectives: a tiny h0-only exchange first (the decoder can
        # start ~15us after the encoder ends), then the bulk sequence
        # exchange, which completes under the decoder prologue. Only the
        # attention (first use at step ~16) needs the bulk data.
        ag0_in = dram.tile([128, KM, B, 2], FP32)
        ag0_out = dram.tile([2, 128, KM, B, 2], FP32)
        nc.sync.dma_start(ag0_in[:, :, :, 0:1], enc_half[:, :, :, 0:1])
        nc.sync.dma_start(ag0_in[:, :, :, 1:2],
                          enc_half[:, :, :, TIN - 1:TIN])
        ag1_in = dram.tile([128, KM, B, TIN], FP32)
        ag1_out = dram.tile([2, 128, KM, B, TIN], FP32)
        nc.sync.dma_start(ag1_in[:], enc_half[:])
        if stub_collectives:
            i0 = ag0_in[:].rearrange("p k b t -> p (k b t)")
            o0 = ag0_out[:].rearrange("g p k b t -> (g p) (k b t)")
            i_f = ag1_in[:].rearrange("p k b t -> p (k b t)")
            o_f = ag1_out[:].rearrange("g p k b t -> (g p) (k b t)")
            for g in range(2):
                nc.sync.dma_start(o0[g * 128:(g + 1) * 128, :], i0)
                nc.sync.dma_start(o_f[g * 128:(g + 1) * 128, :], i_f)
        else:
            collective("AllGather", ALU.bypass,
                       [ag0_in.opt()], [ag0_out.opt()],
                       [[0, 4], [1, 5], [2, 6], [3, 7]])
            collective("AllGather", ALU.bypass,
                       [ag1_in.opt()], [ag1_out.opt()],
                       [[0, 4], [1, 5], [2, 6], [3, 7]])
        ench.release()
        enc_w.release()

        mid = tc.alloc_tile_pool(name="mid", bufs=1)
        # enc_dmaj: [128 d%128, grp, dm, b, s];   d = (grp*KM + dm)*128 + p
        enc_dmaj = mid.tile([128, 2, KM, B, TIN], FP32)
        nc.sync.dma_start(
            enc_dmaj[:],
            ag1_out[:].rearrange("g p k b t -> p g k b t"))
        if debug:
            nc.sync.dma_start(o_enc[:], enc_dmaj[:])
        enc_smaj = mid.tile([128, B, D], BF16)
        # h0 from the small exchange: [fwd h(T-1); bwd h(orig T-1) = its
        # scan column 0]
        ag0_sb = mid.tile([128, 2, KM, B, 2], FP32)
        nc.sync.dma_start(
            ag0_sb[:], ag0_out[:].rearrange("g p k b t -> p g k b t"))
        h_dec = [mid.tile([128, KD, B], HDT, name=f"h_dec{i}")
                 for i in range(2)]
        c_dec = mid.tile([128, KD, B], FP32)
        nc.vector.tensor_copy(h_dec[0][:, 0:KM, :], ag0_sb[:, 0, :, :, 1])
        nc.vector.tensor_copy(h_dec[0][:, KM:2 * KM, :],
                              ag0_sb[:, 1, :, :, 0])
        nc.vector.memset(c_dec[:], 0.0)

        # ---------------- phase 3+4: decoder with interleaved attention ----
        # Token shard is strided: core c attends token positions t = 8*tl + c
        # (tl = 0..15). Position tl's query h_t is ready after decoder step
        # t <= 8*tl + 7, so one attention position rides under each 8-step
        # block of the PE-bound decoder loop (attention is ACT-heavy: 32
        # tanh[128,128] per position, well under 8 steps of PE time). The
        # query is read straight out of dec_outT with a per-core register
        # column offset (partition_id), so no DRAM round-trip is needed.
        dec_outT = mid.tile([128, KD, B, TOUT], FP32)
        # raw scores land in column 8*tl of a TOUT-wide scratch (written at
        # dynamic offset iv0-8; strided-read back after the loop)
        scstore = mid.tile([128, B, TOUT], FP32)
        attnU = mid.tile([128, KD, B, TPC], BF16)
        dn_sb = mid.tile([1, B, TPC], BF16)
        att = tc.alloc_tile_pool(name="att", bufs=3)
        attp = tc.alloc_tile_pool(name="attp", bufs=1, space="PSUM")
        pid = nc.partition_id(engines=(ENG.DVE,))

        # one static mt buffer set, loop-carried: a position's tanh tiles
        # are written (4 per decoder step) during the 8-step block after its
        # query is ready; its score MMs run at the START of the next block,
        # before that block's tanh quartets overwrite the buffers.
        mts_loop = [mid.tile([128, 128], BF16, name=f"mtl{i}")
                    for i in range(B * KD)]

        def attn_qcol(treg, scol):
            # stage the per-core query column (one dynamic DVE read)
            if static_loops:
                # timing-only build: a static query column keeps the
                # register budget flat (values are never executed)
                treg = scol
            qcol = att.tile([128, KD, B], FP32, tag="qcol")
            nc.vector.tensor_copy(
                qcol[:], sq(dec_outT[:, :, :, bass.ds(treg, 1)]))
            return qcol

        def attn_quartet(qcol, j):
            # tanh tiles 4j..4j+3 of the current position: spread across
            # the block's steps so the ACT engine never bursts 32 tanh
            # right when the next block's gate activations need it
            for idx in range(4 * j, 4 * j + 4):
                b, dg = idx // KD, idx % KD
                nc.scalar.activation(
                    mts_loop[idx][:], enc_dmaj[:, dg // KM, dg % KM, b, :],
                    AF.Tanh, bias=qcol[:, dg, b:b + 1])

        def attn_mms(col):
            # score MMs for the position whose tanh tiles are resident:
            # v-stationary, col-tiled 4-up across b, then transpose the
            # (4 x 128) score rows into scstore[col].
            sc_ps = attp.tile([128, 128], FP32, tag="sc")
            for b in range(B):
                for dg in range(KD):
                    nc.tensor.matmul(
                        sc_ps[32 * b:32 * b + 1, :], v_sb[:, dg:dg + 1],
                        mts_loop[b * KD + dg][:], start=(dg == 0),
                        stop=(dg == KD - 1), tile_position=(0, 32 * b))
            sc_sb = att.tile([128, 128], FP32, tag="scsb")
            nc.vector.tensor_copy(sc_sb[:], sc_ps[:])
            scT = attp.tile([128, 128], FP32, tag="scT")
            nc.tensor.transpose(out=scT[:], in_=sc_sb[:], identity=ident[:])
            nc.vector.tensor_copy(
                sq(scstore[:, :, bass.ds(col, 1)]),
                scT[:].rearrange("p (b r) -> p b r", b=B)[:, :, 0:1])

        def dec_steps_block(i0, n):
            for i in range(i0, i0 + n):
                rec_body(i, 1, KD, DEC_GROUPS, whd_sb, xw_d, h_dec,
                         c_dec, dec_outT, MD)

        def emit_smaj():
            # enc_smaj transposes, emitted after the decoder prologue so
            # the PE never stalls on the bulk allgather (enc_dmaj lands
            # during the first ~16 decoder steps)
            for b in range(B):
                for dg in range(KD):
                    tp = attp.tile([128, 128], FP32, tag="scT")
                    nc.tensor.transpose(
                        out=tp[:], in_=enc_dmaj[:, dg // KM, dg % KM, b, :],
                        identity=ident[:])
                    nc.vector.tensor_copy(
                        enc_smaj[:, b, dg * 128:(dg + 1) * 128], tp[:])

        if static_loops:
            qc = [None]
            for i in range(dec_steps):
                if i % 8 == 0 and i >= 16:
                    attn_mms(i - 16)
                if i % 8 == 0 and i >= 8:
                    qc[0] = attn_qcol(i - 8 + pid, i - 8)
                hook = ((lambda j: attn_quartet(qc[0], (i % 8)))
                        if i >= 8 else None)
                rec_body(i, 1, KD, DEC_GROUPS, whd_sb, xw_d, h_dec,
                         c_dec, dec_outT, MD, hook=hook)
                if i == 7:
                    emit_smaj()
        else:
            assert dec_steps % 8 == 0
            # prologue: steps 0..7 plain; 8..15 carry position 0's quartets
            dec_steps_block(0, 8)
            emit_smaj()
            qcol0 = attn_qcol(pid, 0)
            for i in range(8, 16):
                rec_body(i, 1, KD, DEC_GROUPS, whd_sb, xw_d, h_dec,
                         c_dec, dec_outT, MD,
                         hook=(lambda j, _i=i: attn_quartet(qcol0, _i - 8)))

            def dec_body(iv0, unroll):
                attn_mms(iv0 - 16)
                qcol = attn_qcol(iv0 - 8 + pid, None)
                rec_body(iv0, unroll, KD, DEC_GROUPS, whd_sb, xw_d, h_dec,
                         c_dec, dec_outT, MD,
                         hook=lambda j: attn_quartet(qcol, j))

            tc.For_i_unrolled_general(
                16, dec_steps, 1, dec_body,
                max_unroll=8, hint_engines=(ENG.PE,))
        attn_mms(112)                    # position 14
        qcol15 = attn_qcol(120 + pid, 120)
        for j in range(8):
            attn_quartet(qcol15, j)      # position 15
        attn_mms(120)
        if debug:
            nc.sync.dma_start(o_dec[:], dec_outT[:])

        # deferred softmax-numerator + weighted-sum over the 16 positions
        ew = mid.tile([128, B, TPC], BF16)
        nc.scalar.activation(
            ew[:],
            scstore[:].rearrange("p b (q r) -> p b q r", r=8)[:, :, :, 0:1]
            .rearrange("p b q o -> p b (q o)"),
            AF.Exp)
        dn_ps = attp.tile([1, B * TPC], FP32, tag="dn")
        nc.tensor.matmul(dn_ps[:], ones_col[:],
                         ew[:].rearrange("p b t -> p (b t)"),
                         start=True, stop=True)
        nc.vector.tensor_copy(dn_sb[:].rearrange("o b t -> o (b t)"),
                              dn_ps[:])
        for b in range(B):
            au_ps = attp.tile([128, KD, TPC], FP32, tag="au")
            for dg in range(KD):
                nc.tensor.matmul(
                    au_ps[:, dg, :],
                    enc_smaj[:, b, dg * 128:(dg + 1) * 128],
                    ew[:, b, :], start=True, stop=True)
            nc.vector.tensor_copy(attnU[:, :, b, :], au_ps[:])
        attp.release()
        att.release()
        recp.release()

        ag2_in = dram.tile([AGR, TOKC], BF16)
        ag2_out = dram.tile([NC, AGR, TOKC], BF16)
        for k in range(KD):
            nc.sync.dma_start(
                ag2_in[k * 128:(k + 1) * 128, :],
                attnU[:, k, :, :].rearrange("p b t -> p (b t)"))
        nc.sync.dma_start(
            ag2_in[D:D + 1, :], dn_sb[:].rearrange("o b t -> o (b t)"))
        if stub_collectives:
            o_f = ag2_out[:].rearrange("c r t -> (c r) t")
            for g in range(NC):
                nc.sync.dma_start(o_f[g * AGR:(g + 1) * AGR, :], ag2_in[:])
        else:
            collective("AllGather", ALU.bypass,
                       [ag2_in.opt()], [ag2_out.opt()],
                       [list(range(NC))])
        if debug:
            nc.sync.dma_start(o_attn[:], ag2_out[:])
        mid.release()
        dec_w.release()

        # ---------------- phase 5: dense + vocab softmax ------------------
        ph5 = tc.alloc_tile_pool(name="ph5", bufs=1)
        ph5w = tc.alloc_tile_pool(name="ph5w", bufs=8)
        ph5p = tc.alloc_tile_pool(name="ph5p", bufs=4, space="PSUM")
        attn_bf = ph5.tile([128, KD, NTOK], BF16)
        for k in range(KD):
            tmpa = ph5w.tile([128, NC, TOKC], BF16, tag="tmpa")
            nc.sync.dma_start(
                tmpa[:],
                ag2_out[:, k * 128:(k + 1) * 128, :]
                .rearrange("c p t -> p c t"))
            nc.vector.tensor_copy(
                attn_bf[:, k, :].rearrange("p (c t) -> p c t", c=NC),
                tmpa[:])
        # attention-softmax denominators -> per-token reciprocal [128, 4]
        recd_bf = ph5.tile([128, 4], BF16)
        recd = ph5.tile([128, 4], FP32)
        for m in range(4):
            for half in range(2):
                c2 = 2 * m + half
                nc.sync.dma_start(
                    recd_bf[half * 64:(half + 1) * 64, m:m + 1],
                    ag2_out[c2, D:D + 1, :].rearrange("o t -> t o"))
        nc.vector.reciprocal(recd[:], recd_bf[:])

        esum = ph5.tile([128, 4], FP32)
        eprobs = ph5.tile([128, 4, VSH], FP32)
        for m in range(4):
            for n in range(NV):
                dps = ph5p.tile([128, VW], FP32, tag="dps")
                for k in range(KD):
                    wt = ph5w.tile([128, VW], BF16, tag="wo_t")
                    nc.sync.dma_start(
                        wt[:],
                        wo_sh[k * 128:(k + 1) * 128, n * VW:(n + 1) * VW])
                    nc.tensor.matmul(
                        dps[:], attn_bf[:, k, m * 128:(m + 1) * 128],
                        wt[:], start=(k == 0), stop=(k == KD - 1))
                part = ph5w.tile([128, 1], FP32, tag="part")
                lg = ph5w.tile([128, VW], FP32, tag="lg")
                nc.vector.tensor_scalar_mul(lg[:], dps[:], recd[:, m:m + 1])
                nc.scalar.activation(
                    eprobs[:, m, n * VW:(n + 1) * VW], lg[:], AF.Exp,
                    accum_out=part[:, :1])
                if n == 0:
                    nc.vector.tensor_copy(esum[:, m:m + 1], part[:])
                else:
                    nc.vector.tensor_tensor(
                        out=esum[:, m:m + 1], in0=esum[:, m:m + 1],
                        in1=part[:], op=ALU.add)

        ar_in = dram.tile([4, 128], FP32)
        ar_out = dram.tile([4, 128], FP32)
        nc.sync.dma_start(ar_in[:].rearrange("m p -> p m"), esum[:])
        if stub_collectives:
            nc.sync.dma_start(ar_out[:], ar_in[:])
        else:
            collective("AllReduce", ALU.add,
                       [ar_in.opt()], [ar_out.opt()],
                       [list(range(NC))])
        stot = ph5.tile([128, 4], FP32)
        nc.sync.dma_start(stot[:], ar_out[:].rearrange("m p -> p m"))
        nc.vector.reciprocal(stot[:], stot[:])
        for m in range(4):
            for n in range(NV):
                ob = ph5w.tile([128, VW], FP32, tag="ob")
                nc.vector.tensor_scalar_mul(
                    ob[:], eprobs[:, m, n * VW:(n + 1) * VW],
                    stot[:, m:m + 1])
                nc.sync.dma_start(
                    o_probs[m * 128:(m + 1) * 128,
                            n * VW:(n + 1) * VW], ob[:])
        ph5p.release()
        ph5w.release()
        ph5.release()
        dram.release()
        work.release()
        const.release()

    n = legalize_waits(nc)
    if os.environ.get("BASS_LSTM_VERBOSE"):
        print(f"[kernel] legalized {n} waits")
    return nc


_CACHE = {}


def _get_program(debug=False):
    key = ("prog", debug)
    if key not in _CACHE:
        _CACHE[key] = build_program(debug=debug)
    return _CACHE[key]


def pack_gates(w, hper):
    """Keras gate order (i,f,g,o) -> position-major m-tiles: for each
    128-wide state chunk j, the four tiles (i_j, f_j, o_j, g_j)."""
    i, f, g, o = np.split(np.asarray(w), 4, axis=-1)
    gates = (i, f, o, g)
    cols = []
    for j in range(hper // 128):
        for q in range(4):
            cols.append(gates[q][..., j * 128:(j + 1) * 128])
    return np.concatenate(cols, axis=-1)


def q8(w, scale):
    """fp8(e4m3) quantize with prescale (clip to TRN's +-240 max normal)."""
    x = np.asarray(w, np.float32) * scale
    x = np.clip(x, -240.0, 240.0)
    return x.astype(ml_dtypes.float8_e4m3)


def make_in_maps(input_seq, output_seq, enc_emb, dec_emb,
                 Wx_f, Wh_f, b_f, Wx_b, Wh_b, b_b,
                 Wx_d, Wh_d, b_d, attn_scale, Wo, bo):
    bf = ml_dtypes.bfloat16
    Wx_f, Wh_f, b_f = pack_gates(Wx_f, H), pack_gates(Wh_f, H), pack_gates(b_f, H)
    Wx_b, Wh_b, b_b = pack_gates(Wx_b, H), pack_gates(Wh_b, H), pack_gates(b_b, H)
    Wx_d, Wh_d, b_d = pack_gates(Wx_d, D), pack_gates(Wh_d, D), pack_gates(b_d, D)
    assert not np.any(np.asarray(bo)), "bo != 0 not supported by this build"

    def mini_and_idx(emb, seq):
        ids = np.asarray(seq).reshape(-1)              # (b, t) flat
        uniq, inv = np.unique(ids, return_inverse=True)
        mini = np.zeros((NTOK, E), np.float32)
        mini[:len(uniq)] = np.asarray(emb)[uniq]
        idx_col = inv.astype(np.int32).reshape(EM, 128).T.copy()  # [128, EM]
        return mini, idx_col

    enc_mini_f, enc_idx_f = mini_and_idx(enc_emb, input_seq)
    enc_mini_r, enc_idx_r = mini_and_idx(enc_emb,
                                         np.asarray(input_seq)[:, ::-1])
    dec_mini, dec_idx = mini_and_idx(dec_emb, output_seq)

    def bias_cols(bvec, nm):
        # pre-scaled by SC: projections emit SC*(x@Wx + b)
        return (np.asarray(bvec, np.float32) * SC).reshape(nm, 128).T.copy()

    shared = dict(
        dec_mini=dec_mini, dec_idx=dec_idx,
        wx_d=np.asarray(Wx_d).astype(bf), wh_d=q8(Wh_d, SC),
        b_d=bias_cols(b_d, MD),
        v_sc=np.asarray(attn_scale, np.float32).reshape(KD, 128).T
        .astype(bf).copy(),
    )
    fwdw = dict(wx_m=np.asarray(Wx_f).astype(bf),
                wh_m=q8(Wh_f, SC), b_m=bias_cols(b_f, ME))
    bwdw = dict(wx_m=np.asarray(Wx_b).astype(bf),
                wh_m=q8(Wh_b, SC), b_m=bias_cols(b_b, ME))
    Wo_np = np.asarray(Wo)
    in_maps = []
    for c in range(NC):
        m = dict(shared)
        if c < 4:
            m.update(fwdw)
            m.update(enc_mini=enc_mini_f, enc_idx=enc_idx_f)
        else:
            m.update(bwdw)
            m.update(enc_mini=enc_mini_r, enc_idx=enc_idx_r)
        m["wo_sh"] = Wo_np[:, c * VSH:(c + 1) * VSH].astype(bf)
        in_maps.append(m)
    return in_maps


def assemble_output(results):
    out = np.empty((B, TOUT, V), np.float32)
    # gathered token order: r = c2*64 + b*16 + tl ; t = 8*tl + c2
    r = np.arange(NTOK)
    c2, rem = r // TOKC, r % TOKC
    bb, tl = rem // TPC, rem % TPC
    tt = 8 * tl + c2
    for c in range(NC):
        out[bb, tt, c * VSH:(c + 1) * VSH] = results[c]["o_probs"]
    return out


def kernel(**inputs):
    debug = bool(os.environ.get("BASS_LSTM_DEBUG"))
    nc = _get_program(debug=debug)
    in_maps = make_in_maps(**inputs)
    last_exc = None
    for attempt in range(4):
        try:
            res = run_bass_kernel_spmd(nc, in_maps, list(range(NC)))
            break
        except Exception as e:  # transient NRT/axon failures
            last_exc = e
            import time as _t
            _t.sleep(5 * (attempt + 1))
    else:
        raise last_exc
    out = assemble_output(res.results)
    if debug:
        kernel.last_results = res.results
    return out

